# revision 18
# baseline (speedup 1.0000x reference)
"""CascadeMemoryAttention Trainium2 kernel.

Sharding: tensor-parallel over heads (16 heads / 8 cores = 2 heads per core).
Each core computes q/k/v projections for its 2 heads (column slices of the
weights), gated cascade attention, the depthwise causal conv + residual on its
128 channels, then an AllToAll redistributes activations from feature-sharded
to token-sharded so every core applies the full w_o to its 512-token slice.
Host only transposes/slices inputs and concatenates the 8 output slices.

All activations on device are kept feature-major ([feat, token]) so the
host-provided x^T feeds every matmul without any on-device transposes.
Softmax denominators come from a ones-column appended to v (the AV matmul
computes sum(exp) for free).
"""

import os
import sys

import numpy as np
import ml_dtypes

if "/opt/trn_rl_repo" not in sys.path:
    sys.path.insert(0, "/opt/trn_rl_repo")

import concourse.bass as bass
import concourse.bacc as bacc
import concourse.mybir as mybir
import concourse.bass_isa as bass_isa
from concourse import tile
from concourse.bass_utils import run_bass_kernel_spmd

F32 = mybir.dt.float32
BF16 = mybir.dt.bfloat16
AF = mybir.ActivationFunctionType
OP = mybir.AluOpType

NCORES = 8
B, T, C, M = 2, 2048, 1024, 256
H = 16              # total heads
HPC = 2             # heads per core
D = 64              # head dim
S = T + 2 * M       # 2560 kv tokens per batch
NTOK = B * T        # 4096
KVTOK = B * S       # 5120
TCH = 512           # token chunk for attention
NST = S // 128      # 20 s-tiles per batch
CST = T // 128      # 16 chunk s-tiles per batch
SLOT = 2 * (D + 1)  # 130: v slot free-layout [v_h0(64)|1|v_h1(64)|1]
SCALE = 1.0 / 8.0   # 1/sqrt(64)
GATE_REG = 0.01

LAST_RESULT = None  # BassKernelResults of the most recent run (for test.py)

_NC = None


def _build(debug=False):
    nc = bacc.Bacc(num_devices=NCORES)

    xT_d = nc.dram_tensor("xT", [C, NTOK], F32, kind="ExternalInput")
    memT_d = nc.dram_tensor("memT", [C, 2 * 2 * M], F32, kind="ExternalInput")
    wq_d = nc.dram_tensor("wq", [C, 128], F32, kind="ExternalInput")
    wk_d = nc.dram_tensor("wk", [C, 128], F32, kind="ExternalInput")
    wv_d = nc.dram_tensor("wv", [C, 128], F32, kind="ExternalInput")
    wo_d = nc.dram_tensor("wo", [C, C], F32, kind="ExternalInput")
    wqg_d = nc.dram_tensor("wqg", [C, HPC], F32, kind="ExternalInput")
    gb_d = nc.dram_tensor("gb", [HPC, 1], F32, kind="ExternalInput")
    cw0_d = nc.dram_tensor("cw0", [D, 4], F32, kind="ExternalInput")
    cw1_d = nc.dram_tensor("cw1", [D, 4], F32, kind="ExternalInput")
    cb0_d = nc.dram_tensor("cb0", [D, 1], F32, kind="ExternalInput")
    cb1_d = nc.dram_tensor("cb1", [D, 1], F32, kind="ExternalInput")
    mask_d = nc.dram_tensor("mask", [128, 896], BF16, kind="ExternalInput")

    out_d = nc.dram_tensor("out", [NTOK // NCORES, C], F32, kind="ExternalOutput")
    gloss_d = nc.dram_tensor("gloss", [1, 1], F32, kind="ExternalOutput")

    if debug:
        dbg = {
            "d_qT": nc.dram_tensor("d_qT", [128, NTOK], BF16, kind="ExternalOutput"),
            "d_kT": nc.dram_tensor("d_kT", [128, KVTOK], BF16, kind="ExternalOutput"),
            "d_v": nc.dram_tensor(
                "d_v", [128, NST * B * SLOT], BF16, kind="ExternalOutput"
            ),
            "d_y0": nc.dram_tensor("d_y0", [D, NTOK], F32, kind="ExternalOutput"),
            "d_y1": nc.dram_tensor("d_y1", [D, NTOK], F32, kind="ExternalOutput"),
            "d_ysend": nc.dram_tensor(
                "d_ysend", [NCORES, 128, 512], F32, kind="ExternalOutput"
            ),
            "d_yrecv": nc.dram_tensor(
                "d_yrecv", [NCORES, 128, 512], F32, kind="ExternalOutput"
            ),
        }

    with tile.TileContext(nc) as tc:
        with (
            tc.tile_pool(name="const", bufs=1) as cpool,
            tc.tile_pool(name="big", bufs=1) as bpool,
            tc.tile_pool(name="dram", bufs=1, space="DRAM") as dpool,
        ):
            # ---- constants -------------------------------------------------
            wq_t = cpool.tile([128, 8 * 128], F32, tag="wq")
            wk_t = cpool.tile([128, 8 * 128], F32, tag="wk")
            wv_t = cpool.tile([128, 8 * 128], F32, tag="wv")
            wqg_t = cpool.tile([128, 8 * HPC], F32, tag="wqg")
            for w_t, w_d, n in (
                (wq_t, wq_d, 128),
                (wk_t, wk_d, 128),
                (wv_t, wv_d, 128),
                (wqg_t, wqg_d, HPC),
            ):
                nc.sync.dma_start(
                    out=w_t[:].rearrange("p (k n) -> p k n", k=8),
                    in_=w_d[:].rearrange("(k p) n -> p k n", p=128),
                )
            gb_t = cpool.tile([HPC, 1], F32, tag="gb")
            nc.sync.dma_start(out=gb_t[:], in_=gb_d[:])
            cw0_t = cpool.tile([D, 4], F32, tag="cw0")
            nc.sync.dma_start(out=cw0_t[:], in_=cw0_d[:])
            cw1_t = cpool.tile([D, 4], F32, tag="cw1")
            nc.sync.dma_start(out=cw1_t[:], in_=cw1_d[:])
            cb0_t = cpool.tile([D, 1], F32, tag="cb0")
            nc.sync.dma_start(out=cb0_t[:], in_=cb0_d[:])
            cb1_t = cpool.tile([D, 1], F32, tag="cb1")
            nc.sync.dma_start(out=cb1_t[:], in_=cb1_d[:])
            mask_t = cpool.tile([128, 896], BF16, tag="mask")
            nc.sync.dma_start(out=mask_t[:], in_=mask_d[:])
            ones64_t = cpool.tile([D + 1, D], F32, tag="ones64")
            nc.vector.memset(ones64_t[D : D + 1, :], 1.0)

            # ---- persistent activations -----------------------------------
            qT_t = bpool.tile([128, NTOK], BF16, tag="qT")      # [2h*64, tok]
            kT_t = bpool.tile([128, KVTOK], BF16, tag="kT")     # [2h*64, (b,s)]
            v_t = bpool.tile([128, NST * B * SLOT], BF16, tag="v")  # token-major
            g_t = bpool.tile([HPC, NTOK], F32, tag="g")         # sigmoid gates
            g1_t = bpool.tile([1, NTOK], F32, tag="g1")         # row 1 at partition 0
            y0_t = bpool.tile([D, NTOK], F32, tag="y0")         # head-0 attn out
            y1_t = bpool.tile([D, NTOK], F32, tag="y1")         # head-1 attn out

            # ones columns interleaved in v (col 64 / 129 of each 130-slot)
            for st in range(NST * B):
                nc.vector.memset(v_t[:, st * SLOT + D : st * SLOT + D + 1], 1.0)
                nc.vector.memset(
                    v_t[:, st * SLOT + 2 * D + 1 : st * SLOT + 2 * D + 2], 1.0
                )

            # ---- phase 1: projections -------------------------------------
            with (
                tc.tile_pool(name="xt", bufs=2) as xtp,
                tc.tile_pool(name="pj", bufs=2, space="PSUM") as pjp,
            ):
                with nc.named_scope("proj"):
                    # 8 x-blocks of 512 tokens, then 2 memory blocks of 512
                    for blk in range(10):
                        xt_t = xtp.tile([128, 8 * 512], F32, tag="xt")
                        is_mem = blk >= 8
                        mb = blk - 8
                        src = memT_d if is_mem else xT_d
                        c0 = (mb if is_mem else blk) * 512
                        nc.sync.dma_start(
                            out=xt_t[:].rearrange("p (k n) -> p k n", k=8),
                            in_=src[:].rearrange("(k p) n -> p k n", p=128)[
                                :, :, c0 : c0 + 512
                            ],
                        )

                        if is_mem:
                            kcol = mb * S + T          # kv col of this block
                            st0 = mb * NST + CST       # first v stile
                        else:
                            b = blk // 4
                            kcol = b * S + (blk * 512 - b * T)
                            st0 = (blk // 4) * NST + (blk % 4) * 4

                        # k projection
                        ps_k = pjp.tile([128, 512], F32, tag="ps_k")
                        for ci in range(8):
                            nc.tensor.matmul(
                                ps_k[:],
                                wk_t[:, ci * 128 : (ci + 1) * 128],
                                xt_t[:, ci * 512 : (ci + 1) * 512],
                                start=(ci == 0),
                                stop=(ci == 7),
                            )
                        nc.vector.tensor_copy(kT_t[:, kcol : kcol + 512], ps_k[:])

                        if not is_mem:
                            # q projection
                            ps_q = pjp.tile([128, 512], F32, tag="ps_q")
                            for ci in range(8):
                                nc.tensor.matmul(
                                    ps_q[:],
                                    wq_t[:, ci * 128 : (ci + 1) * 128],
                                    xt_t[:, ci * 512 : (ci + 1) * 512],
                                    start=(ci == 0),
                                    stop=(ci == 7),
                                )
                            nc.vector.tensor_copy(
                                qT_t[:, blk * 512 : (blk + 1) * 512], ps_q[:]
                            )
                            # gate logits + sigmoid
                            ps_g = pjp.tile([HPC, 512], F32, tag="ps_g")
                            for ci in range(8):
                                nc.tensor.matmul(
                                    ps_g[:],
                                    wqg_t[:, ci * HPC : (ci + 1) * HPC],
                                    xt_t[:, ci * 512 : (ci + 1) * 512],
                                    start=(ci == 0),
                                    stop=(ci == 7),
                                )
                            nc.scalar.activation(
                                g_t[:, blk * 512 : (blk + 1) * 512],
                                ps_g[:],
                                AF.Sigmoid,
                                bias=gb_t[:, 0:1],
                            )

                        # v projection (token-major), 4 sub-tiles of 128 toks
                        for sub in range(4):
                            ps_v = pjp.tile([128, 128], F32, tag="ps_v")
                            for ci in range(8):
                                nc.tensor.matmul(
                                    ps_v[:],
                                    xt_t[:, ci * 512 + sub * 128 : ci * 512 + (sub + 1) * 128],
                                    wv_t[:, ci * 128 : (ci + 1) * 128],
                                    start=(ci == 0),
                                    stop=(ci == 7),
                                )
                            o = (st0 + sub) * SLOT
                            nc.vector.tensor_copy(v_t[:, o : o + D], ps_v[:, 0:D])
                            nc.vector.tensor_copy(
                                v_t[:, o + D + 1 : o + 2 * D + 1], ps_v[:, D : 2 * D]
                            )

            # move gate row 1 to a partition-0 tile (partition_broadcast
            # sources must start at partition 0)
            nc.sync.dma_start(out=g1_t[:], in_=g_t[1:2, :])

            # ---- phase 2: attention ---------------------------------------
            with (
                tc.tile_pool(name="ps_s", bufs=2, space="PSUM") as psp,
                tc.tile_pool(name="ps_av", bufs=2, space="PSUM") as avp,
                tc.tile_pool(name="ps_br", bufs=2, space="PSUM") as brp,
                tc.tile_pool(name="expp", bufs=6) as expp,
                tc.tile_pool(name="nrm", bufs=2) as nrm,
            ):
                with nc.named_scope("attn"):
                    for b in range(B):
                        for h in range(HPC):
                            y_t = (y0_t, y1_t)[h]
                            hr = slice(h * D, (h + 1) * D)
                            for tch in range(T // TCH):
                                q_ap = qT_t[hr, b * T + tch * TCH : b * T + (tch + 1) * TCH]
                                vis = 4 * (tch + 1)
                                psum_c = avp.tile([D + 1, TCH], F32, tag="avc")
                                psum_m = avp.tile([D + 1, TCH], F32, tag="avm")

                                def stile(st, psum, first, last):
                                    ps = psp.tile([128, TCH], F32, tag="ps")
                                    nc.tensor.matmul(
                                        ps[:],
                                        kT_t[hr, b * S + st * 128 : b * S + (st + 1) * 128],
                                        q_ap,
                                        start=True,
                                        stop=True,
                                    )
                                    et = expp.tile([128, TCH], BF16, tag="et")
                                    nc.scalar.activation(et[:], ps[:], AF.Exp, scale=SCALE)
                                    delta = st * 128 - tch * TCH
                                    if 0 <= delta < TCH and st < CST:
                                        nc.gpsimd.tensor_mul(
                                            et[:],
                                            et[:],
                                            mask_t[:, 384 - delta : 384 - delta + TCH],
                                        )
                                    o = (b * NST + st) * SLOT + h * (D + 1)
                                    nc.tensor.matmul(
                                        psum[:],
                                        v_t[:, o : o + D + 1],
                                        et[:],
                                        start=first,
                                        stop=last,
                                    )

                                for st in range(vis):
                                    stile(st, psum_c, st == 0, st == vis - 1)
                                for j, st in enumerate(range(CST, NST)):
                                    stile(st, psum_m, j == 0, j == 3)

                                # normalize + gate combine
                                # scr rows 0..63: scratch; row 64: l then 1/l
                                scr = nrm.tile([D + 1, TCH], F32, tag="scr")
                                nc.vector.tensor_copy(
                                    scr[D : D + 1, :], psum_m[D : D + 1, :]
                                )
                                nc.vector.tensor_add(
                                    scr[D : D + 1, :], psum_c[D : D + 1, :], scr[D : D + 1, :]
                                )
                                nc.vector.reciprocal(scr[D : D + 1, :], scr[D : D + 1, :])
                                # broadcast 1/l (partition 64) to partitions 0..63
                                # via a K=1 outer-product matmul into PSUM
                                brinv = brp.tile([D, TCH], F32, tag="brinv")
                                nc.tensor.matmul(
                                    brinv[:],
                                    ones64_t[D : D + 1, :],
                                    scr[D : D + 1, :],
                                    start=True,
                                    stop=True,
                                )
                                bg = nrm.tile([D, TCH], F32, tag="bg")
                                g_src = (g_t, g1_t)[h]
                                nc.gpsimd.partition_broadcast(
                                    bg[:],
                                    g_src[0:1, b * T + tch * TCH : b * T + (tch + 1) * TCH],
                                    channels=D,
                                )
                                nc.vector.tensor_mul(scr[0:D, :], psum_m[0:D, :], bg[:])
                                nc.vector.tensor_add(scr[0:D, :], psum_c[0:D, :], scr[0:D, :])
                                nc.vector.tensor_mul(
                                    y_t[:, b * T + tch * TCH : b * T + (tch + 1) * TCH],
                                    brinv[:],
                                    scr[0:D, :],
                                )

            # ---- phase 3+4: conv + residual, scatter to A2A staging -------
            y_send = dpool.tile([NCORES, 128, 512], F32, tag="ysend")
            y_recv = dpool.tile([NCORES, 128, 512], F32, tag="yrecv")
            with tc.tile_pool(name="cv", bufs=2) as cvp:
                with nc.named_scope("conv"):
                    for h in range(HPC):
                        y_t = (y0_t, y1_t)[h]
                        cw = (cw0_t, cw1_t)[h]
                        cb = (cb0_t, cb1_t)[h][:, 0:1]
                        for b in range(B):
                            t0 = b * T
                            y2 = cvp.tile([D, T], F32, tag="y2")
                            nc.vector.tensor_scalar_mul(
                                y2[:], y_t[:, t0 : t0 + T], cw[:, 3:4]
                            )
                            for k in range(3):
                                sh = 3 - k
                                nc.vector.scalar_tensor_tensor(
                                    y2[:, sh:T],
                                    y_t[:, t0 : t0 + T - sh],
                                    cw[:, k : k + 1],
                                    y2[:, sh:T],
                                    OP.mult,
                                    OP.add,
                                )
                            nc.vector.scalar_tensor_tensor(
                                y2[:],
                                y2[:],
                                cb,
                                y_t[:, t0 : t0 + T],
                                OP.add,
                                OP.add,
                            )
                            # scatter: y_send[dest, h*64:(h+1)*64, :] over 4 dests
                            nc.sync.dma_start(
                                out=y_send[
                                    b * 4 : (b + 1) * 4, h * D : (h + 1) * D, :
                                ].rearrange("c p n -> p c n"),
                                in_=y2[:].rearrange("p (c n) -> p c n", c=4),
                            )

            with nc.named_scope("a2a"):
                nc.gpsimd.collective_compute(
                    "AllToAll",
                    OP.bypass,
                    replica_groups=[list(range(NCORES))],
                    ins=[y_send[:].opt()],
                    outs=[y_recv[:].opt()],
                )

            # ---- phase 5: output projection on own 512-token slice --------
            with (
                tc.tile_pool(name="yr", bufs=8) as yrp,
                tc.tile_pool(name="wo", bufs=2) as wop,
                tc.tile_pool(name="po", bufs=8, space="PSUM") as pop,
                tc.tile_pool(name="os", bufs=3) as osp,
            ):
                with nc.named_scope("oproj"):
                    yr = []
                    for ci in range(8):
                        t_ = yrp.tile([128, 512], F32, tag="yr")
                        nc.sync.dma_start(out=t_[:], in_=y_recv[ci, :, :])
                        yr.append(t_)
                    psums = {}
                    for tt in range(4):
                        for oc in range(2):
                            psums[(tt, oc)] = pop.tile(
                                [128, 512], F32, tag="po", name=f"po{tt}{oc}"
                            )
                    for ci in range(8):
                        wo_t = wop.tile([128, C], F32, tag="wo")
                        nc.sync.dma_start(
                            out=wo_t[:], in_=wo_d[ci * 128 : (ci + 1) * 128, :]
                        )
                        for tt in range(4):
                            for oc in range(2):
                                nc.tensor.matmul(
                                    psums[(tt, oc)][:],
                                    yr[ci][:, tt * 128 : (tt + 1) * 128],
                                    wo_t[:, oc * 512 : (oc + 1) * 512],
                                    start=(ci == 0),
                                    stop=(ci == 7),
                                )
                    for tt in range(4):
                        for oc in range(2):
                            ot = osp.tile([128, 512], F32, tag="os")
                            nc.vector.tensor_copy(ot[:], psums[(tt, oc)][:])
                            nc.sync.dma_start(
                                out=out_d[tt * 128 : (tt + 1) * 128, oc * 512 : (oc + 1) * 512],
                                in_=ot[:],
                            )

            if debug:
                nc.sync.dma_start(out=dbg["d_qT"][:], in_=qT_t[:])
                nc.sync.dma_start(out=dbg["d_kT"][:], in_=kT_t[:])
                nc.sync.dma_start(out=dbg["d_v"][:], in_=v_t[:])
                nc.sync.dma_start(out=dbg["d_y0"][:], in_=y0_t[:])
                nc.sync.dma_start(out=dbg["d_y1"][:], in_=y1_t[:])
                nc.sync.dma_start(out=dbg["d_ysend"][:], in_=y_send[:])
                nc.sync.dma_start(out=dbg["d_yrecv"][:], in_=y_recv[:])

            # ---- phase 6: gate regularization partial sum -----------------
            gs = cpool.tile([HPC, 1], F32, tag="gs")
            gsr = cpool.tile([HPC, 1], F32, tag="gsr")
            nc.vector.tensor_reduce(gs[:], g_t[:], axis=mybir.AxisListType.X, op=OP.add)
            nc.gpsimd.partition_all_reduce(
                gsr[:], gs[:], channels=HPC, reduce_op=bass_isa.ReduceOp.add
            )
            nc.sync.dma_start(out=gloss_d[:], in_=gsr[0:1, :])

    nc.compile()
    return nc


def _get_nc(debug=False):
    global _NC
    if _NC is None:
        _NC = _build(debug=debug)
    return _NC


_SHIMMED = False


def _install_trace_shims():
    """Enable NTFF tracing under axon in this image.

    The image's ``antenv`` package lacks ``axon_hooks`` (so bass_utils'
    trace path degrades); register an equivalent module backed by the
    ctypes hook from trn_agent_boot. Also stub the S3 artifact upload
    (zero-egress container).
    """
    global _SHIMMED
    if _SHIMMED:
        return
    _SHIMMED = True
    import types

    import concourse.bass_utils as bu

    bu.upload_artifacts = lambda tmpdir: tmpdir

    try:
        import antenv.axon_hooks  # noqa: F401
        return  # real module exists
    except ImportError:
        pass
    try:
        from trn_agent_boot.trn_boot import _ntff_profile_via_ctypes
    except ImportError:
        return
    hook = _ntff_profile_via_ctypes("/opt/axon/libaxon_pjrt.so")
    mod = types.ModuleType("antenv.axon_hooks")
    mod.get_axon_ntff_profile_hook = lambda: hook
    mod.set_axon_ntff_profile_hook = lambda h: None
    sys.modules["antenv.axon_hooks"] = mod


def kernel(x, forward_memory, reverse_memory, w_q, w_k, w_v, w_o,
           gate_w, gate_b, conv_w, conv_b):
    f32 = np.float32
    x = np.asarray(x, f32)
    fm = np.asarray(forward_memory, f32)
    rm = np.asarray(reverse_memory, f32)
    w_q = np.asarray(w_q, f32)
    w_k = np.asarray(w_k, f32)
    w_v = np.asarray(w_v, f32)
    w_o = np.asarray(w_o, f32)
    gate_w = np.asarray(gate_w, f32)
    gate_b = np.asarray(gate_b, f32)
    conv_w = np.asarray(conv_w, f32)
    conv_b = np.asarray(conv_b, f32)

    xT = np.ascontiguousarray(x.reshape(NTOK, C).T)
    mem = np.concatenate([fm[0], rm[0], fm[1], rm[1]], axis=0)  # [4M, C]
    memT = np.ascontiguousarray(mem.T)
    wqg = w_q @ gate_w  # [C, H]

    ii = np.arange(128)[:, None]
    cc = np.arange(896)[None, :]
    mask = (ii <= cc - 384).astype(ml_dtypes.bfloat16)

    cw = conv_w[:, 0, :]  # [4, C]

    in_maps = []
    for c in range(NCORES):
        cs = slice(c * 128, (c + 1) * 128)
        in_maps.append({
            "xT": xT,
            "memT": memT,
            "wq": np.ascontiguousarray(w_q[:, cs]),
            "wk": np.ascontiguousarray(w_k[:, cs]),
            "wv": np.ascontiguousarray(w_v[:, cs]),
            "wo": w_o,
            "wqg": np.ascontiguousarray(wqg[:, c * HPC : (c + 1) * HPC]),
            "gb": np.ascontiguousarray(
                gate_b[c * HPC : (c + 1) * HPC].reshape(HPC, 1)
            ),
            "cw0": np.ascontiguousarray(cw[:, c * 128 : c * 128 + D].T),
            "cw1": np.ascontiguousarray(cw[:, c * 128 + D : (c + 1) * 128].T),
            "cb0": np.ascontiguousarray(conv_b[c * 128 : c * 128 + D].reshape(D, 1)),
            "cb1": np.ascontiguousarray(
                conv_b[c * 128 + D : (c + 1) * 128].reshape(D, 1)
            ),
            "mask": mask,
        })

    trace = bool(int(os.environ.get("KERNEL_TRACE", "0")))
    if trace:
        _install_trace_shims()

    nc = _get_nc(debug=bool(int(os.environ.get("KERNEL_DEBUG", "0"))))
    res = run_bass_kernel_spmd(
        nc,
        in_maps,
        core_ids=list(range(NCORES)),
        trace=trace,
    )
    global LAST_RESULT
    LAST_RESULT = res

    out = np.concatenate([r["out"] for r in res.results], axis=0)
    out = out.reshape(B, T, C)
    gtot = sum(float(r["gloss"][0, 0]) for r in res.results)
    gloss = np.float32(GATE_REG * gtot / (B * T * H))
    return out, gloss


# revision 19
# speedup vs baseline: 1.4653x; 1.4653x over previous
"""CascadeMemoryAttention Trainium2 kernel.

Sharding: tensor-parallel over heads (16 heads / 8 cores = 2 heads per core).
Each core computes q/k/v projections for its 2 heads (column slices of the
weights), gated cascade attention, the depthwise causal conv + residual on its
128 channels, then an AllToAll redistributes activations from feature-sharded
to token-sharded so every core applies the full w_o to its 512-token slice.
Host only transposes/slices inputs and concatenates the 8 output slices.

All activations on device are kept feature-major ([feat, token]) so the
host-provided x^T feeds every matmul without any on-device transposes.
Softmax denominators come from a ones-column appended to v (the AV matmul
computes sum(exp) for free).
"""

import os
import sys

import numpy as np
import ml_dtypes

if "/opt/trn_rl_repo" not in sys.path:
    sys.path.insert(0, "/opt/trn_rl_repo")

import concourse.bass as bass
import concourse.bacc as bacc
import concourse.mybir as mybir
import concourse.bass_isa as bass_isa
from concourse import tile
from concourse.bass_utils import run_bass_kernel_spmd

F32 = mybir.dt.float32
BF16 = mybir.dt.bfloat16
AF = mybir.ActivationFunctionType
OP = mybir.AluOpType

NCORES = 8
B, T, C, M = 2, 2048, 1024, 256
H = 16              # total heads
HPC = 2             # heads per core
D = 64              # head dim
S = T + 2 * M       # 2560 kv tokens per batch
NTOK = B * T        # 4096
KVTOK = B * S       # 5120
TCH = 512           # token chunk for attention
NST = S // 128      # 20 s-tiles per batch
CST = T // 128      # 16 chunk s-tiles per batch
SLOT = 2 * (D + 1)  # 130: v slot free-layout [v_h0(64)|1|v_h1(64)|1]
SCALE = 1.0 / 8.0   # 1/sqrt(64)
GATE_REG = 0.01

LAST_RESULT = None  # BassKernelResults of the most recent run (for test.py)

_NC = None


def _build(debug=False):
    nc = bacc.Bacc(num_devices=NCORES)

    xT_d = nc.dram_tensor("xT", [C, NTOK], BF16, kind="ExternalInput")
    memT_d = nc.dram_tensor("memT", [C, 2 * 2 * M], BF16, kind="ExternalInput")
    wq_d = nc.dram_tensor("wq", [C, 128], BF16, kind="ExternalInput")
    wk_d = nc.dram_tensor("wk", [C, 128], BF16, kind="ExternalInput")
    wv_d = nc.dram_tensor("wv", [C, 128], BF16, kind="ExternalInput")
    wo_d = nc.dram_tensor("wo", [C, C], BF16, kind="ExternalInput")
    wqg_d = nc.dram_tensor("wqg", [C, HPC], BF16, kind="ExternalInput")
    gb_d = nc.dram_tensor("gb", [HPC, 1], F32, kind="ExternalInput")
    cw0_d = nc.dram_tensor("cw0", [D, 4], F32, kind="ExternalInput")
    cw1_d = nc.dram_tensor("cw1", [D, 4], F32, kind="ExternalInput")
    cb0_d = nc.dram_tensor("cb0", [D, 1], F32, kind="ExternalInput")
    cb1_d = nc.dram_tensor("cb1", [D, 1], F32, kind="ExternalInput")
    mask_d = nc.dram_tensor("mask", [128, 896], BF16, kind="ExternalInput")

    out_d = nc.dram_tensor("out", [NTOK // NCORES, C], F32, kind="ExternalOutput")
    gloss_d = nc.dram_tensor("gloss", [1, 1], F32, kind="ExternalOutput")

    if debug:
        dbg = {
            "d_qT": nc.dram_tensor("d_qT", [128, NTOK], BF16, kind="ExternalOutput"),
            "d_kT": nc.dram_tensor("d_kT", [128, KVTOK], BF16, kind="ExternalOutput"),
            "d_v": nc.dram_tensor(
                "d_v", [128, NST * B * SLOT], BF16, kind="ExternalOutput"
            ),
            "d_y0": nc.dram_tensor("d_y0", [D, NTOK], F32, kind="ExternalOutput"),
            "d_y1": nc.dram_tensor("d_y1", [D, NTOK], F32, kind="ExternalOutput"),
            "d_ysend": nc.dram_tensor(
                "d_ysend", [NCORES, 128, 512], BF16, kind="ExternalOutput"
            ),
            "d_yrecv": nc.dram_tensor(
                "d_yrecv", [NCORES, 128, 512], BF16, kind="ExternalOutput"
            ),
        }

    with tile.TileContext(nc) as tc:
        with (
            tc.tile_pool(name="const", bufs=1) as cpool,
            tc.tile_pool(name="big", bufs=1) as bpool,
            tc.tile_pool(name="dram", bufs=1, space="DRAM") as dpool,
        ):
            # ---- constants -------------------------------------------------
            wq_t = cpool.tile([128, 8 * 128], BF16, tag="wq")
            wk_t = cpool.tile([128, 8 * 128], BF16, tag="wk")
            wv_t = cpool.tile([128, 8 * 128], BF16, tag="wv")
            wqg_t = cpool.tile([128, 8 * HPC], BF16, tag="wqg")
            for w_t, w_d, n in (
                (wq_t, wq_d, 128),
                (wk_t, wk_d, 128),
                (wv_t, wv_d, 128),
                (wqg_t, wqg_d, HPC),
            ):
                nc.sync.dma_start(
                    out=w_t[:].rearrange("p (k n) -> p k n", k=8),
                    in_=w_d[:].rearrange("(k p) n -> p k n", p=128),
                )
            gb_t = cpool.tile([HPC, 1], F32, tag="gb")
            nc.sync.dma_start(out=gb_t[:], in_=gb_d[:])
            cw0_t = cpool.tile([D, 4], F32, tag="cw0")
            nc.sync.dma_start(out=cw0_t[:], in_=cw0_d[:])
            cw1_t = cpool.tile([D, 4], F32, tag="cw1")
            nc.sync.dma_start(out=cw1_t[:], in_=cw1_d[:])
            cb0_t = cpool.tile([D, 1], F32, tag="cb0")
            nc.sync.dma_start(out=cb0_t[:], in_=cb0_d[:])
            cb1_t = cpool.tile([D, 1], F32, tag="cb1")
            nc.sync.dma_start(out=cb1_t[:], in_=cb1_d[:])
            mask_t = cpool.tile([128, 896], BF16, tag="mask")
            nc.sync.dma_start(out=mask_t[:], in_=mask_d[:])
            ones64_t = cpool.tile([D + 1, D], F32, tag="ones64")
            nc.vector.memset(ones64_t[D : D + 1, :], 1.0)

            # ---- persistent activations -----------------------------------
            qT_t = bpool.tile([128, NTOK], BF16, tag="qT")      # [2h*64, tok]
            kT_t = bpool.tile([128, KVTOK], BF16, tag="kT")     # [2h*64, (b,s)]
            v_t = bpool.tile([128, NST * B * SLOT], BF16, tag="v")  # token-major
            g_t = bpool.tile([HPC, NTOK], F32, tag="g")         # sigmoid gates
            g1_t = bpool.tile([1, NTOK], F32, tag="g1")         # row 1 at partition 0
            y0_t = bpool.tile([D, NTOK], F32, tag="y0")         # head-0 attn out
            y1_t = bpool.tile([D, NTOK], F32, tag="y1")         # head-1 attn out

            # ones columns interleaved in v (col 64 / 129 of each 130-slot)
            for st in range(NST * B):
                nc.vector.memset(v_t[:, st * SLOT + D : st * SLOT + D + 1], 1.0)
                nc.vector.memset(
                    v_t[:, st * SLOT + 2 * D + 1 : st * SLOT + 2 * D + 2], 1.0
                )

            # ---- phase 1: projections -------------------------------------
            with (
                tc.tile_pool(name="xt", bufs=2) as xtp,
                tc.tile_pool(name="pj", bufs=2, space="PSUM") as pjp,
            ):
                with nc.named_scope("proj"):
                    # 8 x-blocks of 512 tokens, then 2 memory blocks of 512
                    for blk in range(10):
                        xt_t = xtp.tile([128, 8 * 512], BF16, tag="xt")
                        is_mem = blk >= 8
                        mb = blk - 8
                        src = memT_d if is_mem else xT_d
                        c0 = (mb if is_mem else blk) * 512
                        nc.sync.dma_start(
                            out=xt_t[:].rearrange("p (k n) -> p k n", k=8),
                            in_=src[:].rearrange("(k p) n -> p k n", p=128)[
                                :, :, c0 : c0 + 512
                            ],
                        )

                        if is_mem:
                            kcol = mb * S + T          # kv col of this block
                            st0 = mb * NST + CST       # first v stile
                        else:
                            b = blk // 4
                            kcol = b * S + (blk * 512 - b * T)
                            st0 = (blk // 4) * NST + (blk % 4) * 4

                        # k projection
                        ps_k = pjp.tile([128, 512], F32, tag="ps_k")
                        for ci in range(8):
                            nc.tensor.matmul(
                                ps_k[:],
                                wk_t[:, ci * 128 : (ci + 1) * 128],
                                xt_t[:, ci * 512 : (ci + 1) * 512],
                                start=(ci == 0),
                                stop=(ci == 7),
                            )
                        nc.vector.tensor_copy(kT_t[:, kcol : kcol + 512], ps_k[:])

                        if not is_mem:
                            # q projection
                            ps_q = pjp.tile([128, 512], F32, tag="ps_q")
                            for ci in range(8):
                                nc.tensor.matmul(
                                    ps_q[:],
                                    wq_t[:, ci * 128 : (ci + 1) * 128],
                                    xt_t[:, ci * 512 : (ci + 1) * 512],
                                    start=(ci == 0),
                                    stop=(ci == 7),
                                )
                            nc.vector.tensor_copy(
                                qT_t[:, blk * 512 : (blk + 1) * 512], ps_q[:]
                            )
                            # gate logits + sigmoid
                            ps_g = pjp.tile([HPC, 512], F32, tag="ps_g")
                            for ci in range(8):
                                nc.tensor.matmul(
                                    ps_g[:],
                                    wqg_t[:, ci * HPC : (ci + 1) * HPC],
                                    xt_t[:, ci * 512 : (ci + 1) * 512],
                                    start=(ci == 0),
                                    stop=(ci == 7),
                                )
                            nc.scalar.activation(
                                g_t[:, blk * 512 : (blk + 1) * 512],
                                ps_g[:],
                                AF.Sigmoid,
                                bias=gb_t[:, 0:1],
                            )

                        # v projection (token-major), 4 sub-tiles of 128 toks
                        for sub in range(4):
                            ps_v = pjp.tile([128, 128], F32, tag="ps_v")
                            for ci in range(8):
                                nc.tensor.matmul(
                                    ps_v[:],
                                    xt_t[:, ci * 512 + sub * 128 : ci * 512 + (sub + 1) * 128],
                                    wv_t[:, ci * 128 : (ci + 1) * 128],
                                    start=(ci == 0),
                                    stop=(ci == 7),
                                )
                            o = (st0 + sub) * SLOT
                            nc.vector.tensor_copy(v_t[:, o : o + D], ps_v[:, 0:D])
                            nc.vector.tensor_copy(
                                v_t[:, o + D + 1 : o + 2 * D + 1], ps_v[:, D : 2 * D]
                            )

            # move gate row 1 to a partition-0 tile (partition_broadcast
            # sources must start at partition 0)
            nc.sync.dma_start(out=g1_t[:], in_=g_t[1:2, :])

            # ---- phase 2: attention ---------------------------------------
            with (
                tc.tile_pool(name="ps_s", bufs=2, space="PSUM") as psp,
                tc.tile_pool(name="ps_av", bufs=2, space="PSUM") as avp,
                tc.tile_pool(name="ps_br", bufs=2, space="PSUM") as brp,
                tc.tile_pool(name="expp", bufs=6) as expp,
                tc.tile_pool(name="nrm", bufs=2) as nrm,
            ):
                with nc.named_scope("attn"):
                    for b in range(B):
                        for h in range(HPC):
                            y_t = (y0_t, y1_t)[h]
                            hr = slice(h * D, (h + 1) * D)
                            for tch in range(T // TCH):
                                q_ap = qT_t[hr, b * T + tch * TCH : b * T + (tch + 1) * TCH]
                                vis = 4 * (tch + 1)
                                psum_c = avp.tile([D + 1, TCH], F32, tag="avc")
                                psum_m = avp.tile([D + 1, TCH], F32, tag="avm")

                                def stile(st, psum, first, last):
                                    ps = psp.tile([128, TCH], F32, tag="ps")
                                    nc.tensor.matmul(
                                        ps[:],
                                        kT_t[hr, b * S + st * 128 : b * S + (st + 1) * 128],
                                        q_ap,
                                        start=True,
                                        stop=True,
                                    )
                                    et = expp.tile([128, TCH], BF16, tag="et")
                                    nc.scalar.activation(et[:], ps[:], AF.Exp, scale=SCALE)
                                    delta = st * 128 - tch * TCH
                                    if 0 <= delta < TCH and st < CST:
                                        nc.gpsimd.tensor_mul(
                                            et[:],
                                            et[:],
                                            mask_t[:, 384 - delta : 384 - delta + TCH],
                                        )
                                    o = (b * NST + st) * SLOT + h * (D + 1)
                                    nc.tensor.matmul(
                                        psum[:],
                                        v_t[:, o : o + D + 1],
                                        et[:],
                                        start=first,
                                        stop=last,
                                    )

                                for st in range(vis):
                                    stile(st, psum_c, st == 0, st == vis - 1)
                                for j, st in enumerate(range(CST, NST)):
                                    stile(st, psum_m, j == 0, j == 3)

                                # normalize + gate combine
                                # scr rows 0..63: scratch; row 64: l then 1/l
                                scr = nrm.tile([D + 1, TCH], F32, tag="scr")
                                nc.vector.tensor_copy(
                                    scr[D : D + 1, :], psum_m[D : D + 1, :]
                                )
                                nc.vector.tensor_add(
                                    scr[D : D + 1, :], psum_c[D : D + 1, :], scr[D : D + 1, :]
                                )
                                nc.vector.reciprocal(scr[D : D + 1, :], scr[D : D + 1, :])
                                # broadcast 1/l (partition 64) to partitions 0..63
                                # via a K=1 outer-product matmul into PSUM
                                brinv = brp.tile([D, TCH], F32, tag="brinv")
                                nc.tensor.matmul(
                                    brinv[:],
                                    ones64_t[D : D + 1, :],
                                    scr[D : D + 1, :],
                                    start=True,
                                    stop=True,
                                )
                                bg = nrm.tile([D, TCH], F32, tag="bg")
                                g_src = (g_t, g1_t)[h]
                                nc.gpsimd.partition_broadcast(
                                    bg[:],
                                    g_src[0:1, b * T + tch * TCH : b * T + (tch + 1) * TCH],
                                    channels=D,
                                )
                                nc.vector.tensor_mul(scr[0:D, :], psum_m[0:D, :], bg[:])
                                nc.vector.tensor_add(scr[0:D, :], psum_c[0:D, :], scr[0:D, :])
                                nc.vector.tensor_mul(
                                    y_t[:, b * T + tch * TCH : b * T + (tch + 1) * TCH],
                                    brinv[:],
                                    scr[0:D, :],
                                )

            # ---- phase 3+4: conv + residual, scatter to A2A staging -------
            y_send = dpool.tile([NCORES, 128, 512], BF16, tag="ysend")
            y_recv = dpool.tile([NCORES, 128, 512], BF16, tag="yrecv")
            with tc.tile_pool(name="cv", bufs=2) as cvp:
                with nc.named_scope("conv"):
                    for h in range(HPC):
                        y_t = (y0_t, y1_t)[h]
                        cw = (cw0_t, cw1_t)[h]
                        cb = (cb0_t, cb1_t)[h][:, 0:1]
                        for b in range(B):
                            t0 = b * T
                            y2 = cvp.tile([D, T], F32, tag="y2")
                            nc.vector.tensor_scalar_mul(
                                y2[:], y_t[:, t0 : t0 + T], cw[:, 3:4]
                            )
                            for k in range(3):
                                sh = 3 - k
                                nc.vector.scalar_tensor_tensor(
                                    y2[:, sh:T],
                                    y_t[:, t0 : t0 + T - sh],
                                    cw[:, k : k + 1],
                                    y2[:, sh:T],
                                    OP.mult,
                                    OP.add,
                                )
                            nc.vector.scalar_tensor_tensor(
                                y2[:],
                                y2[:],
                                cb,
                                y_t[:, t0 : t0 + T],
                                OP.add,
                                OP.add,
                            )
                            # scatter: y_send[dest, h*64:(h+1)*64, :] over 4 dests
                            nc.gpsimd.dma_start(
                                out=y_send[
                                    b * 4 : (b + 1) * 4, h * D : (h + 1) * D, :
                                ].rearrange("c p n -> p c n"),
                                in_=y2[:].rearrange("p (c n) -> p c n", c=4),
                            )

            with nc.named_scope("a2a"):
                nc.gpsimd.collective_compute(
                    "AllToAll",
                    OP.bypass,
                    replica_groups=[list(range(NCORES))],
                    ins=[y_send[:].opt()],
                    outs=[y_recv[:].opt()],
                )

            # ---- phase 5: output projection on own 512-token slice --------
            with (
                tc.tile_pool(name="yr", bufs=8) as yrp,
                tc.tile_pool(name="wo", bufs=2) as wop,
                tc.tile_pool(name="po", bufs=8, space="PSUM") as pop,
                tc.tile_pool(name="os", bufs=3) as osp,
            ):
                with nc.named_scope("oproj"):
                    yr = []
                    for ci in range(8):
                        t_ = yrp.tile([128, 512], BF16, tag="yr")
                        nc.sync.dma_start(out=t_[:], in_=y_recv[ci, :, :])
                        yr.append(t_)
                    psums = {}
                    for tt in range(4):
                        for oc in range(2):
                            psums[(tt, oc)] = pop.tile(
                                [128, 512], F32, tag="po", name=f"po{tt}{oc}"
                            )
                    for ci in range(8):
                        wo_t = wop.tile([128, C], BF16, tag="wo")
                        nc.sync.dma_start(
                            out=wo_t[:], in_=wo_d[ci * 128 : (ci + 1) * 128, :]
                        )
                        for tt in range(4):
                            for oc in range(2):
                                nc.tensor.matmul(
                                    psums[(tt, oc)][:],
                                    yr[ci][:, tt * 128 : (tt + 1) * 128],
                                    wo_t[:, oc * 512 : (oc + 1) * 512],
                                    start=(ci == 0),
                                    stop=(ci == 7),
                                )
                    for tt in range(4):
                        for oc in range(2):
                            ot = osp.tile([128, 512], F32, tag="os")
                            nc.vector.tensor_copy(ot[:], psums[(tt, oc)][:])
                            nc.sync.dma_start(
                                out=out_d[tt * 128 : (tt + 1) * 128, oc * 512 : (oc + 1) * 512],
                                in_=ot[:],
                            )

            if debug:
                nc.sync.dma_start(out=dbg["d_qT"][:], in_=qT_t[:])
                nc.sync.dma_start(out=dbg["d_kT"][:], in_=kT_t[:])
                nc.sync.dma_start(out=dbg["d_v"][:], in_=v_t[:])
                nc.sync.dma_start(out=dbg["d_y0"][:], in_=y0_t[:])
                nc.sync.dma_start(out=dbg["d_y1"][:], in_=y1_t[:])
                nc.sync.dma_start(out=dbg["d_ysend"][:], in_=y_send[:])
                nc.sync.dma_start(out=dbg["d_yrecv"][:], in_=y_recv[:])

            # ---- phase 6: gate regularization partial sum -----------------
            gs = cpool.tile([HPC, 1], F32, tag="gs")
            gsr = cpool.tile([HPC, 1], F32, tag="gsr")
            nc.vector.tensor_reduce(gs[:], g_t[:], axis=mybir.AxisListType.X, op=OP.add)
            nc.gpsimd.partition_all_reduce(
                gsr[:], gs[:], channels=HPC, reduce_op=bass_isa.ReduceOp.add
            )
            nc.sync.dma_start(out=gloss_d[:], in_=gsr[0:1, :])

    nc.compile()
    return nc


def _get_nc(debug=False):
    global _NC
    if _NC is None:
        _NC = _build(debug=debug)
    return _NC


_SHIMMED = False


def _install_trace_shims():
    """Enable NTFF tracing under axon in this image.

    The image's ``antenv`` package lacks ``axon_hooks`` (so bass_utils'
    trace path degrades); register an equivalent module backed by the
    ctypes hook from trn_agent_boot. Also stub the S3 artifact upload
    (zero-egress container).
    """
    global _SHIMMED
    if _SHIMMED:
        return
    _SHIMMED = True
    import types

    import concourse.bass_utils as bu

    bu.upload_artifacts = lambda tmpdir: tmpdir

    try:
        import antenv.axon_hooks  # noqa: F401
        return  # real module exists
    except ImportError:
        pass
    try:
        from trn_agent_boot.trn_boot import _ntff_profile_via_ctypes
    except ImportError:
        return
    hook = _ntff_profile_via_ctypes("/opt/axon/libaxon_pjrt.so")
    mod = types.ModuleType("antenv.axon_hooks")
    mod.get_axon_ntff_profile_hook = lambda: hook
    mod.set_axon_ntff_profile_hook = lambda h: None
    sys.modules["antenv.axon_hooks"] = mod


def kernel(x, forward_memory, reverse_memory, w_q, w_k, w_v, w_o,
           gate_w, gate_b, conv_w, conv_b):
    f32 = np.float32
    x = np.asarray(x, f32)
    fm = np.asarray(forward_memory, f32)
    rm = np.asarray(reverse_memory, f32)
    w_q = np.asarray(w_q, f32)
    w_k = np.asarray(w_k, f32)
    w_v = np.asarray(w_v, f32)
    w_o = np.asarray(w_o, f32)
    gate_w = np.asarray(gate_w, f32)
    gate_b = np.asarray(gate_b, f32)
    conv_w = np.asarray(conv_w, f32)
    conv_b = np.asarray(conv_b, f32)

    bf16 = ml_dtypes.bfloat16
    xT = np.ascontiguousarray(x.reshape(NTOK, C).T).astype(bf16)
    mem = np.concatenate([fm[0], rm[0], fm[1], rm[1]], axis=0)  # [4M, C]
    memT = np.ascontiguousarray(mem.T).astype(bf16)
    wqg = w_q @ gate_w  # [C, H]

    ii = np.arange(128)[:, None]
    cc = np.arange(896)[None, :]
    mask = (ii <= cc - 384).astype(ml_dtypes.bfloat16)

    cw = conv_w[:, 0, :]  # [4, C]

    in_maps = []
    for c in range(NCORES):
        cs = slice(c * 128, (c + 1) * 128)
        in_maps.append({
            "xT": xT,
            "memT": memT,
            "wq": np.ascontiguousarray(w_q[:, cs]).astype(bf16),
            "wk": np.ascontiguousarray(w_k[:, cs]).astype(bf16),
            "wv": np.ascontiguousarray(w_v[:, cs]).astype(bf16),
            "wo": w_o.astype(bf16),
            "wqg": np.ascontiguousarray(wqg[:, c * HPC : (c + 1) * HPC]).astype(bf16),
            "gb": np.ascontiguousarray(
                gate_b[c * HPC : (c + 1) * HPC].reshape(HPC, 1)
            ),
            "cw0": np.ascontiguousarray(cw[:, c * 128 : c * 128 + D].T),
            "cw1": np.ascontiguousarray(cw[:, c * 128 + D : (c + 1) * 128].T),
            "cb0": np.ascontiguousarray(conv_b[c * 128 : c * 128 + D].reshape(D, 1)),
            "cb1": np.ascontiguousarray(
                conv_b[c * 128 + D : (c + 1) * 128].reshape(D, 1)
            ),
            "mask": mask,
        })

    trace = bool(int(os.environ.get("KERNEL_TRACE", "0")))
    if trace:
        _install_trace_shims()

    nc = _get_nc(debug=bool(int(os.environ.get("KERNEL_DEBUG", "0"))))
    res = run_bass_kernel_spmd(
        nc,
        in_maps,
        core_ids=list(range(NCORES)),
        trace=trace,
    )
    global LAST_RESULT
    LAST_RESULT = res

    out = np.concatenate([r["out"] for r in res.results], axis=0)
    out = out.reshape(B, T, C)
    gtot = sum(float(r["gloss"][0, 0]) for r in res.results)
    gloss = np.float32(GATE_REG * gtot / (B * T * H))
    return out, gloss


# revision 21
# speedup vs baseline: 1.5046x; 1.0268x over previous
"""CascadeMemoryAttention Trainium2 kernel.

Sharding: tensor-parallel over heads (16 heads / 8 cores = 2 heads per core).
Each core computes q/k/v projections for its 2 heads (column slices of the
weights), gated cascade attention, the depthwise causal conv + residual on its
128 channels, then an AllToAll redistributes activations from feature-sharded
to token-sharded so every core applies the full w_o to its 512-token slice.
Host only transposes/slices inputs and concatenates the 8 output slices.

All activations on device are kept feature-major ([feat, token]) so the
host-provided x^T feeds every matmul without any on-device transposes.
Softmax denominators come from a ones-column appended to v (the AV matmul
computes sum(exp) for free).
"""

import os
import sys

import numpy as np
import ml_dtypes

if "/opt/trn_rl_repo" not in sys.path:
    sys.path.insert(0, "/opt/trn_rl_repo")

import concourse.bass as bass
import concourse.bacc as bacc
import concourse.mybir as mybir
import concourse.bass_isa as bass_isa
from concourse import tile
from concourse.bass_utils import run_bass_kernel_spmd

F32 = mybir.dt.float32
BF16 = mybir.dt.bfloat16
AF = mybir.ActivationFunctionType
OP = mybir.AluOpType

NCORES = 8
B, T, C, M = 2, 2048, 1024, 256
H = 16              # total heads
HPC = 2             # heads per core
D = 64              # head dim
S = T + 2 * M       # 2560 kv tokens per batch
NTOK = B * T        # 4096
KVTOK = B * S       # 5120
TCH = 512           # token chunk for attention
NST = S // 128      # 20 s-tiles per batch
CST = T // 128      # 16 chunk s-tiles per batch
SLOT = 2 * (D + 1)  # 130: v slot free-layout [v_h0(64)|1|v_h1(64)|1]
SCALE = 1.0 / 8.0   # 1/sqrt(64)
GATE_REG = 0.01

LAST_RESULT = None  # BassKernelResults of the most recent run (for test.py)

_NC = None


def _build(debug=False):
    nc = bacc.Bacc(num_devices=NCORES)

    xT_d = nc.dram_tensor("xT", [C, NTOK], BF16, kind="ExternalInput")
    memT_d = nc.dram_tensor("memT", [C, 2 * 2 * M], BF16, kind="ExternalInput")
    wq_d = nc.dram_tensor("wq", [C, 128], BF16, kind="ExternalInput")
    wk_d = nc.dram_tensor("wk", [C, 128], BF16, kind="ExternalInput")
    wv_d = nc.dram_tensor("wv", [C, 128], BF16, kind="ExternalInput")
    wo_d = nc.dram_tensor("wo", [C, C], BF16, kind="ExternalInput")
    wqg_d = nc.dram_tensor("wqg", [C, HPC], BF16, kind="ExternalInput")
    gb_d = nc.dram_tensor("gb", [HPC, 1], F32, kind="ExternalInput")
    cw0_d = nc.dram_tensor("cw0", [D, 4], F32, kind="ExternalInput")
    cw1_d = nc.dram_tensor("cw1", [D, 4], F32, kind="ExternalInput")
    cb0_d = nc.dram_tensor("cb0", [D, 1], F32, kind="ExternalInput")
    cb1_d = nc.dram_tensor("cb1", [D, 1], F32, kind="ExternalInput")
    mask_d = nc.dram_tensor("mask", [128, 896], BF16, kind="ExternalInput")

    out_d = nc.dram_tensor("out", [NTOK // NCORES, C], F32, kind="ExternalOutput")
    gloss_d = nc.dram_tensor("gloss", [1, 1], F32, kind="ExternalOutput")

    if debug:
        dbg = {
            "d_qT": nc.dram_tensor("d_qT", [128, NTOK], BF16, kind="ExternalOutput"),
            "d_kT": nc.dram_tensor("d_kT", [128, KVTOK], BF16, kind="ExternalOutput"),
            "d_v": nc.dram_tensor(
                "d_v", [128, NST * B * SLOT], BF16, kind="ExternalOutput"
            ),
            "d_y0": nc.dram_tensor("d_y0", [D, NTOK], F32, kind="ExternalOutput"),
            "d_y1": nc.dram_tensor("d_y1", [D, NTOK], F32, kind="ExternalOutput"),
            "d_ysend": nc.dram_tensor(
                "d_ysend", [NCORES, 128, 512], BF16, kind="ExternalOutput"
            ),
            "d_yrecv": nc.dram_tensor(
                "d_yrecv", [NCORES, 128, 512], BF16, kind="ExternalOutput"
            ),
        }

    with tile.TileContext(nc) as tc:
        with (
            tc.tile_pool(name="const", bufs=1) as cpool,
            tc.tile_pool(name="big", bufs=1) as bpool,
            tc.tile_pool(name="dram", bufs=1, space="DRAM") as dpool,
        ):
            # ---- constants -------------------------------------------------
            wq_t = cpool.tile([128, 8 * 128], BF16, tag="wq")
            wk_t = cpool.tile([128, 8 * 128], BF16, tag="wk")
            wv_t = cpool.tile([128, 8 * 128], BF16, tag="wv")
            wqg_t = cpool.tile([128, 8 * HPC], BF16, tag="wqg")
            for w_t, w_d, n in (
                (wq_t, wq_d, 128),
                (wk_t, wk_d, 128),
                (wv_t, wv_d, 128),
                (wqg_t, wqg_d, HPC),
            ):
                nc.sync.dma_start(
                    out=w_t[:].rearrange("p (k n) -> p k n", k=8),
                    in_=w_d[:].rearrange("(k p) n -> p k n", p=128),
                )
            gb_t = cpool.tile([HPC, 1], F32, tag="gb")
            nc.sync.dma_start(out=gb_t[:], in_=gb_d[:])
            cw0_t = cpool.tile([D, 4], F32, tag="cw0")
            nc.sync.dma_start(out=cw0_t[:], in_=cw0_d[:])
            cw1_t = cpool.tile([D, 4], F32, tag="cw1")
            nc.sync.dma_start(out=cw1_t[:], in_=cw1_d[:])
            cb0_t = cpool.tile([D, 1], F32, tag="cb0")
            nc.sync.dma_start(out=cb0_t[:], in_=cb0_d[:])
            cb1_t = cpool.tile([D, 1], F32, tag="cb1")
            nc.sync.dma_start(out=cb1_t[:], in_=cb1_d[:])
            mask_t = cpool.tile([128, 896], BF16, tag="mask")
            nc.sync.dma_start(out=mask_t[:], in_=mask_d[:])
            ones64_t = cpool.tile([D + 1, D], BF16, tag="ones64")
            nc.vector.memset(ones64_t[D : D + 1, :], 1.0)

            # ---- persistent activations -----------------------------------
            qT_t = bpool.tile([128, NTOK], BF16, tag="qT")      # [2h*64, tok]
            kT_t = bpool.tile([128, KVTOK], BF16, tag="kT")     # [2h*64, (b,s)]
            v_t = bpool.tile([128, NST * B * SLOT], BF16, tag="v")  # token-major
            g_t = bpool.tile([HPC, NTOK], F32, tag="g")         # sigmoid gates
            g1_t = bpool.tile([1, NTOK], F32, tag="g1")         # row 1 at partition 0
            y0_t = bpool.tile([D, NTOK], F32, tag="y0")         # head-0 attn out
            y1_t = bpool.tile([D, NTOK], F32, tag="y1")         # head-1 attn out

            # ones columns interleaved in v (col 64 / 129 of each 130-slot)
            for st in range(NST * B):
                nc.vector.memset(v_t[:, st * SLOT + D : st * SLOT + D + 1], 1.0)
                nc.vector.memset(
                    v_t[:, st * SLOT + 2 * D + 1 : st * SLOT + 2 * D + 2], 1.0
                )

            # ---- phase 1: projections -------------------------------------
            with (
                tc.tile_pool(name="xt", bufs=2) as xtp,
                tc.tile_pool(name="pj", bufs=2, space="PSUM") as pjp,
            ):
                with nc.named_scope("proj"):
                    # 8 x-blocks of 512 tokens, then 2 memory blocks of 512
                    for blk in (8, 9, 0, 1, 2, 3, 4, 5, 6, 7):
                        xt_t = xtp.tile([128, 8 * 512], BF16, tag="xt")
                        is_mem = blk >= 8
                        mb = blk - 8
                        src = memT_d if is_mem else xT_d
                        c0 = (mb if is_mem else blk) * 512
                        nc.sync.dma_start(
                            out=xt_t[:].rearrange("p (k n) -> p k n", k=8),
                            in_=src[:].rearrange("(k p) n -> p k n", p=128)[
                                :, :, c0 : c0 + 512
                            ],
                        )

                        if is_mem:
                            kcol = mb * S + T          # kv col of this block
                            st0 = mb * NST + CST       # first v stile
                        else:
                            b = blk // 4
                            kcol = b * S + (blk * 512 - b * T)
                            st0 = (blk // 4) * NST + (blk % 4) * 4

                        # k projection
                        ps_k = pjp.tile([128, 512], F32, tag="ps_k")
                        for ci in range(8):
                            nc.tensor.matmul(
                                ps_k[:],
                                wk_t[:, ci * 128 : (ci + 1) * 128],
                                xt_t[:, ci * 512 : (ci + 1) * 512],
                                start=(ci == 0),
                                stop=(ci == 7),
                            )
                        nc.vector.tensor_copy(kT_t[:, kcol : kcol + 512], ps_k[:])

                        if not is_mem:
                            # q projection
                            ps_q = pjp.tile([128, 512], F32, tag="ps_q")
                            for ci in range(8):
                                nc.tensor.matmul(
                                    ps_q[:],
                                    wq_t[:, ci * 128 : (ci + 1) * 128],
                                    xt_t[:, ci * 512 : (ci + 1) * 512],
                                    start=(ci == 0),
                                    stop=(ci == 7),
                                )
                            nc.vector.tensor_copy(
                                qT_t[:, blk * 512 : (blk + 1) * 512], ps_q[:]
                            )
                            # gate logits + sigmoid
                            ps_g = pjp.tile([HPC, 512], F32, tag="ps_g")
                            for ci in range(8):
                                nc.tensor.matmul(
                                    ps_g[:],
                                    wqg_t[:, ci * HPC : (ci + 1) * HPC],
                                    xt_t[:, ci * 512 : (ci + 1) * 512],
                                    start=(ci == 0),
                                    stop=(ci == 7),
                                )
                            nc.scalar.activation(
                                g_t[:, blk * 512 : (blk + 1) * 512],
                                ps_g[:],
                                AF.Sigmoid,
                                bias=gb_t[:, 0:1],
                            )

                        # v projection (token-major), 4 sub-tiles of 128 toks
                        for sub in range(4):
                            ps_v = pjp.tile([128, 128], F32, tag="ps_v")
                            for ci in range(8):
                                nc.tensor.matmul(
                                    ps_v[:],
                                    xt_t[:, ci * 512 + sub * 128 : ci * 512 + (sub + 1) * 128],
                                    wv_t[:, ci * 128 : (ci + 1) * 128],
                                    start=(ci == 0),
                                    stop=(ci == 7),
                                )
                            o = (st0 + sub) * SLOT
                            nc.vector.tensor_copy(v_t[:, o : o + D], ps_v[:, 0:D])
                            nc.vector.tensor_copy(
                                v_t[:, o + D + 1 : o + 2 * D + 1], ps_v[:, D : 2 * D]
                            )

            # move gate row 1 to a partition-0 tile (partition_broadcast
            # sources must start at partition 0)
            nc.sync.dma_start(out=g1_t[:], in_=g_t[1:2, :])

            # ---- phase 2: attention + fused conv/scatter ------------------
            y_send = dpool.tile([NCORES, 128, 512], BF16, tag="ysend")
            y_recv = dpool.tile([NCORES, 128, 512], BF16, tag="yrecv")
            with (
                tc.tile_pool(name="ps_s", bufs=3, space="PSUM") as psp,
                tc.tile_pool(name="ps_av", bufs=2, space="PSUM") as avp,
                tc.tile_pool(name="ps_br", bufs=1, space="PSUM") as brp,
                tc.tile_pool(name="expp", bufs=6) as expp,
                tc.tile_pool(name="nrm", bufs=2) as nrm,
                tc.tile_pool(name="cv", bufs=2) as cvp,
            ):
                with nc.named_scope("attn"):
                    for b in range(B):
                        for h in range(HPC):
                            y_t = (y0_t, y1_t)[h]
                            hr = slice(h * D, (h + 1) * D)
                            for tch in range(T // TCH):
                                q_ap = qT_t[hr, b * T + tch * TCH : b * T + (tch + 1) * TCH]
                                vis = 4 * (tch + 1)
                                psum_c = avp.tile([D + 1, TCH], F32, tag="avc")
                                psum_m = avp.tile([D + 1, TCH], F32, tag="avm")

                                def stile(st, psum, first, last):
                                    ps = psp.tile([128, TCH], F32, tag="ps")
                                    nc.tensor.matmul(
                                        ps[:],
                                        kT_t[hr, b * S + st * 128 : b * S + (st + 1) * 128],
                                        q_ap,
                                        start=True,
                                        stop=True,
                                    )
                                    et = expp.tile([128, TCH], BF16, tag="et")
                                    nc.scalar.activation(et[:], ps[:], AF.Exp, scale=SCALE)
                                    delta = st * 128 - tch * TCH
                                    if 0 <= delta < TCH and st < CST:
                                        nc.gpsimd.tensor_mul(
                                            et[:],
                                            et[:],
                                            mask_t[:, 384 - delta : 384 - delta + TCH],
                                        )
                                    o = (b * NST + st) * SLOT + h * (D + 1)
                                    nc.tensor.matmul(
                                        psum[:],
                                        v_t[:, o : o + D + 1],
                                        et[:],
                                        start=first,
                                        stop=last,
                                    )

                                for st in range(vis):
                                    stile(st, psum_c, st == 0, st == vis - 1)
                                for j, st in enumerate(range(CST, NST)):
                                    stile(st, psum_m, j == 0, j == 3)

                                # normalize + gate combine
                                # scr rows 0..63: scratch; row 64: l then 1/l
                                scr = nrm.tile([D + 1, TCH], F32, tag="scr")
                                nc.vector.tensor_copy(
                                    scr[D : D + 1, :], psum_m[D : D + 1, :]
                                )
                                nc.vector.tensor_add(
                                    scr[D : D + 1, :], psum_c[D : D + 1, :], scr[D : D + 1, :]
                                )
                                nc.vector.reciprocal(scr[D : D + 1, :], scr[D : D + 1, :])
                                rb16 = nrm.tile([D + 1, TCH], BF16, tag="rb16")
                                nc.vector.tensor_copy(
                                    rb16[D : D + 1, :], scr[D : D + 1, :]
                                )
                                # broadcast 1/l (partition 64) to partitions 0..63
                                # via a K=1 outer-product matmul into PSUM
                                brinv = brp.tile([D, TCH], F32, tag="brinv")
                                nc.tensor.matmul(
                                    brinv[:],
                                    ones64_t[D : D + 1, :],
                                    rb16[D : D + 1, :],
                                    start=True,
                                    stop=True,
                                )
                                bg = nrm.tile([D, TCH], F32, tag="bg")
                                g_src = (g_t, g1_t)[h]
                                nc.gpsimd.partition_broadcast(
                                    bg[:],
                                    g_src[0:1, b * T + tch * TCH : b * T + (tch + 1) * TCH],
                                    channels=D,
                                )
                                nc.vector.tensor_mul(scr[0:D, :], psum_m[0:D, :], bg[:])
                                nc.vector.tensor_add(scr[0:D, :], psum_c[0:D, :], scr[0:D, :])
                                nc.vector.tensor_mul(
                                    y_t[:, b * T + tch * TCH : b * T + (tch + 1) * TCH],
                                    brinv[:],
                                    scr[0:D, :],
                                )

                            # conv + residual + A2A scatter for this (b, h),
                            # overlapped with the next head's attention
                            cw = (cw0_t, cw1_t)[h]
                            cb = (cb0_t, cb1_t)[h][:, 0:1]
                            t0 = b * T
                            y2 = cvp.tile([D, T], F32, tag="y2", name=f"y2_{b}_{h}")
                            nc.vector.tensor_scalar_mul(
                                y2[:], y_t[:, t0 : t0 + T], cw[:, 3:4]
                            )
                            for k in range(3):
                                sh = 3 - k
                                nc.vector.scalar_tensor_tensor(
                                    y2[:, sh:T],
                                    y_t[:, t0 : t0 + T - sh],
                                    cw[:, k : k + 1],
                                    y2[:, sh:T],
                                    OP.mult,
                                    OP.add,
                                )
                            nc.vector.scalar_tensor_tensor(
                                y2[:],
                                y2[:],
                                cb,
                                y_t[:, t0 : t0 + T],
                                OP.add,
                                OP.add,
                            )
                            nc.gpsimd.dma_start(
                                out=y_send[
                                    b * 4 : (b + 1) * 4, h * D : (h + 1) * D, :
                                ].rearrange("c p n -> p c n"),
                                in_=y2[:].rearrange("p (c n) -> p c n", c=4),
                            )

            with nc.named_scope("a2a"):
                nc.gpsimd.collective_compute(
                    "AllToAll",
                    OP.bypass,
                    replica_groups=[list(range(NCORES))],
                    ins=[y_send[:].opt()],
                    outs=[y_recv[:].opt()],
                )

            # ---- phase 5: output projection on own 512-token slice --------
            with (
                tc.tile_pool(name="yr", bufs=8) as yrp,
                tc.tile_pool(name="wo", bufs=2) as wop,
                tc.tile_pool(name="po", bufs=8, space="PSUM") as pop,
                tc.tile_pool(name="os", bufs=3) as osp,
            ):
                with nc.named_scope("oproj"):
                    yr = []
                    for ci in range(8):
                        t_ = yrp.tile([128, 512], BF16, tag="yr")
                        nc.sync.dma_start(out=t_[:], in_=y_recv[ci, :, :])
                        yr.append(t_)
                    psums = {}
                    for tt in range(4):
                        for oc in range(2):
                            psums[(tt, oc)] = pop.tile(
                                [128, 512], F32, tag="po", name=f"po{tt}{oc}"
                            )
                    for ci in range(8):
                        wo_t = wop.tile([128, C], BF16, tag="wo")
                        nc.sync.dma_start(
                            out=wo_t[:], in_=wo_d[ci * 128 : (ci + 1) * 128, :]
                        )
                        for tt in range(4):
                            for oc in range(2):
                                nc.tensor.matmul(
                                    psums[(tt, oc)][:],
                                    yr[ci][:, tt * 128 : (tt + 1) * 128],
                                    wo_t[:, oc * 512 : (oc + 1) * 512],
                                    start=(ci == 0),
                                    stop=(ci == 7),
                                )
                    for tt in range(4):
                        for oc in range(2):
                            ot = osp.tile([128, 512], F32, tag="os")
                            nc.vector.tensor_copy(ot[:], psums[(tt, oc)][:])
                            nc.sync.dma_start(
                                out=out_d[tt * 128 : (tt + 1) * 128, oc * 512 : (oc + 1) * 512],
                                in_=ot[:],
                            )

            if debug:
                nc.sync.dma_start(out=dbg["d_qT"][:], in_=qT_t[:])
                nc.sync.dma_start(out=dbg["d_kT"][:], in_=kT_t[:])
                nc.sync.dma_start(out=dbg["d_v"][:], in_=v_t[:])
                nc.sync.dma_start(out=dbg["d_y0"][:], in_=y0_t[:])
                nc.sync.dma_start(out=dbg["d_y1"][:], in_=y1_t[:])
                nc.sync.dma_start(out=dbg["d_ysend"][:], in_=y_send[:])
                nc.sync.dma_start(out=dbg["d_yrecv"][:], in_=y_recv[:])

            # ---- phase 6: gate regularization partial sum -----------------
            gs = cpool.tile([HPC, 1], F32, tag="gs")
            gsr = cpool.tile([HPC, 1], F32, tag="gsr")
            nc.vector.tensor_reduce(gs[:], g_t[:], axis=mybir.AxisListType.X, op=OP.add)
            nc.gpsimd.partition_all_reduce(
                gsr[:], gs[:], channels=HPC, reduce_op=bass_isa.ReduceOp.add
            )
            nc.sync.dma_start(out=gloss_d[:], in_=gsr[0:1, :])

    nc.compile()
    return nc


def _get_nc(debug=False):
    global _NC
    if _NC is None:
        _NC = _build(debug=debug)
    return _NC


_SHIMMED = False


def _install_trace_shims():
    """Enable NTFF tracing under axon in this image.

    The image's ``antenv`` package lacks ``axon_hooks`` (so bass_utils'
    trace path degrades); register an equivalent module backed by the
    ctypes hook from trn_agent_boot. Also stub the S3 artifact upload
    (zero-egress container).
    """
    global _SHIMMED
    if _SHIMMED:
        return
    _SHIMMED = True
    import types

    import concourse.bass_utils as bu

    bu.upload_artifacts = lambda tmpdir: tmpdir

    try:
        import antenv.axon_hooks  # noqa: F401
        return  # real module exists
    except ImportError:
        pass
    try:
        from trn_agent_boot.trn_boot import _ntff_profile_via_ctypes
    except ImportError:
        return
    hook = _ntff_profile_via_ctypes("/opt/axon/libaxon_pjrt.so")
    mod = types.ModuleType("antenv.axon_hooks")
    mod.get_axon_ntff_profile_hook = lambda: hook
    mod.set_axon_ntff_profile_hook = lambda h: None
    sys.modules["antenv.axon_hooks"] = mod


def kernel(x, forward_memory, reverse_memory, w_q, w_k, w_v, w_o,
           gate_w, gate_b, conv_w, conv_b):
    f32 = np.float32
    x = np.asarray(x, f32)
    fm = np.asarray(forward_memory, f32)
    rm = np.asarray(reverse_memory, f32)
    w_q = np.asarray(w_q, f32)
    w_k = np.asarray(w_k, f32)
    w_v = np.asarray(w_v, f32)
    w_o = np.asarray(w_o, f32)
    gate_w = np.asarray(gate_w, f32)
    gate_b = np.asarray(gate_b, f32)
    conv_w = np.asarray(conv_w, f32)
    conv_b = np.asarray(conv_b, f32)

    bf16 = ml_dtypes.bfloat16
    xT = np.ascontiguousarray(x.reshape(NTOK, C).T).astype(bf16)
    mem = np.concatenate([fm[0], rm[0], fm[1], rm[1]], axis=0)  # [4M, C]
    memT = np.ascontiguousarray(mem.T).astype(bf16)
    wqg = w_q @ gate_w  # [C, H]

    ii = np.arange(128)[:, None]
    cc = np.arange(896)[None, :]
    mask = (ii <= cc - 384).astype(ml_dtypes.bfloat16)

    cw = conv_w[:, 0, :]  # [4, C]

    in_maps = []
    for c in range(NCORES):
        cs = slice(c * 128, (c + 1) * 128)
        in_maps.append({
            "xT": xT,
            "memT": memT,
            "wq": np.ascontiguousarray(w_q[:, cs]).astype(bf16),
            "wk": np.ascontiguousarray(w_k[:, cs]).astype(bf16),
            "wv": np.ascontiguousarray(w_v[:, cs]).astype(bf16),
            "wo": w_o.astype(bf16),
            "wqg": np.ascontiguousarray(wqg[:, c * HPC : (c + 1) * HPC]).astype(bf16),
            "gb": np.ascontiguousarray(
                gate_b[c * HPC : (c + 1) * HPC].reshape(HPC, 1)
            ),
            "cw0": np.ascontiguousarray(cw[:, c * 128 : c * 128 + D].T),
            "cw1": np.ascontiguousarray(cw[:, c * 128 + D : (c + 1) * 128].T),
            "cb0": np.ascontiguousarray(conv_b[c * 128 : c * 128 + D].reshape(D, 1)),
            "cb1": np.ascontiguousarray(
                conv_b[c * 128 + D : (c + 1) * 128].reshape(D, 1)
            ),
            "mask": mask,
        })

    trace = bool(int(os.environ.get("KERNEL_TRACE", "0")))
    if trace:
        _install_trace_shims()

    nc = _get_nc(debug=bool(int(os.environ.get("KERNEL_DEBUG", "0"))))
    res = run_bass_kernel_spmd(
        nc,
        in_maps,
        core_ids=list(range(NCORES)),
        trace=trace,
    )
    global LAST_RESULT
    LAST_RESULT = res

    out = np.concatenate([r["out"] for r in res.results], axis=0)
    out = out.reshape(B, T, C)
    gtot = sum(float(r["gloss"][0, 0]) for r in res.results)
    gloss = np.float32(GATE_REG * gtot / (B * T * H))
    return out, gloss
